# revision 1
# baseline (speedup 1.0000x reference)
"""Trainium2 Bass kernel for the CNN_LSTM forecast problem (B=256, H=2048,
80 teacher-forced + 80 autoregressive steps) on 8 NeuronCores.

Strategy (self-contained; shapes hardcoded):
  - Hidden dimension sharded 8 ways: each core owns 256 hidden units
    (1024 gate rows); W_hh/W_eff slices stay SBUF-resident in fp16.
  - Full batch on every core, split into two independent halves of 128 that
    are interleaved step-wise so each half's AllGather (h.T slice exchange)
    overlaps the other half's matmuls.
  - Transposed layout gates.T [gate_rows, batch]: ACT applies per-partition
    bias for free, h.T slices feed the AllGather directly, no transposes.
  - Phase 2 feedback folded via W_eff = W_hh + W_ih @ W_lin (the scalar
    output re-entry becomes part of the recurrence matrix), so the output
    projection is off the critical path; per-step partial predictions
    (W_lin_slice @ h_slice) are summed across cores on the host.
"""
import numpy as np

import concourse.bass as bass
import concourse.mybir as mybir
import concourse.tile as tile
from concourse import bacc
from concourse.bass2jax import _bass_exec_p, install_neuronx_cc_hook, partition_id_tensor
from concourse.bass_interp import get_hw_module

F32 = mybir.dt.float32
F16 = mybir.dt.float16
AF = mybir.ActivationFunctionType

NCORES = 8
B = 256
NB = 2
BH = B // NB
H = 2048
HS = H // NCORES
GS = 4 * HS
KCH = H // 128
MCH = GS // 128


def _build(T, FUT, repeat=1, warm_mms=40):
    S = T + FUT
    nc = bacc.Bacc("TRN2", target_bir_lowering=False, debug=False,
                   enable_asserts=False, num_devices=NCORES)

    w1_d = nc.dram_tensor("w1sb", [128, KCH, GS], F16, kind="ExternalInput").ap()
    w2_d = nc.dram_tensor("w2sb", [128, KCH, GS], F16, kind="ExternalInput").ap()
    wih_d = nc.dram_tensor("wih", [1, GS], F16, kind="ExternalInput").ap()
    wlin_d = nc.dram_tensor("wlin", [128, 2], F16, kind="ExternalInput").ap()
    b1_d = nc.dram_tensor("b1", [128, MCH], F32, kind="ExternalInput").ap()
    b2_d = nc.dram_tensor("b2", [128, MCH], F32, kind="ExternalInput").ap()
    xt_d = nc.dram_tensor("xt16", [T, B], F16, kind="ExternalInput").ap()
    preds_d = nc.dram_tensor("preds", [S, B], F32, kind="ExternalOutput").ap()

    cc_in = [[nc.dram_tensor(f"cc_in{h}_{i}", [HS, BH], F16, kind="Internal").ap()
              for i in range(2)] for h in range(NB)]
    cc_out = [[nc.dram_tensor(f"cc_out{h}_{i}", [H, BH], F16, kind="Internal",
                              addr_space="Shared").ap()
               for i in range(2)] for h in range(NB)]
    rg = [list(range(NCORES))]

    with tile.TileContext(nc) as tc:
        with (
            tc.tile_pool(name="singles", bufs=1) as singles,
            tc.tile_pool(name="state", bufs=1) as state,
            tc.tile_pool(name="gact", bufs=24) as gact_pool,
            tc.tile_pool(name="h16p", bufs=6) as h16_pool,
            tc.tile_pool(name="xst", bufs=8) as xst_pool,
            tc.tile_pool(name="predrow", bufs=6) as pred_pool,
            tc.tile_pool(name="ps", bufs=8, space="PSUM") as ps,
        ):
            w1 = singles.tile([128, KCH, GS], F16)
            w2 = singles.tile([128, KCH, GS], F16)
            wih = singles.tile([1, GS], F16)
            wlin = singles.tile([128, 2], F16)
            b1 = singles.tile([128, MCH], F32)
            b2 = singles.tile([128, MCH], F32)
            nc.sync.dma_start(w1[:], w1_d[:])
            nc.sync.dma_start(w2[:], w2_d[:])
            nc.sync.dma_start(wih[:], wih_d[:])
            nc.sync.dma_start(wlin[:], wlin_d[:])
            nc.sync.dma_start(b1[:], b1_d[:])
            nc.sync.dma_start(b2[:], b2_d[:])

            hT = [state.tile([128, KCH, BH], F16, name=f"hT{i}", tag=f"hT{i}")
                  for i in range(NB)]
            c_st = [state.tile([128, 2, BH], F32, name=f"c{i}", tag=f"c{i}")
                    for i in range(NB)]
            wsc = state.tile([128, BH], F16, name="wsc", tag="wsc")
            nc.vector.memset(wsc[:], 0.0)
            pending_in = [None, None]

            def flush_pending(h):
                if pending_in[h] is None:
                    return
                src = pending_in[h]
                pending_in[h] = None
                kk = KCH // 2
                for i in range(2):
                    nc.sync.dma_start(hT[h][:, i * kk:(i + 1) * kk, :],
                                      src[:, i * kk:(i + 1) * kk, :])

            def emit_half_step(h, t):
                phase1 = t < T
                wsb = w1 if phase1 else w2
                bias = b1 if phase1 else b2
                buf = t % 2
                boff = h * BH

                flush_pending(h)
                if phase1:
                    xst = xst_pool.tile([1, BH], F16, tag="xs")
                    nc.sync.dma_start(xst[:], xt_d[t:t + 1, boff:boff + BH])

                aux = ps.tile([128, 4, BH], F32, tag="ps")
                if warm_mms:
                    # keep the PE HAM clock-gate open while our allgather lands
                    for i in range(warm_mms):
                        nc.tensor.matmul(aux[:, 0, :], wsb[:, 0, 0:128], wsc[:],
                                         start=(i == 0),
                                         stop=(i == warm_mms - 1))

                # hidden-chunk-grouped m order: chunk 0's gates finish first so
                # its ACT/DVE/h16/cc_in chain overlaps chunk 1's matmuls
                morder = [0, 2, 4, 6, 1, 3, 5, 7]
                gacts = {}
                h16 = h16_pool.tile([128, 2, BH], F16, tag="h16")
                cc_dst = cc_in[h][buf].rearrange("(c p) b -> p c b", p=128)
                for mq in range(2):
                    ps_t = ps.tile([128, 4, BH], F32, tag="ps")
                    for sub in range(4):
                        m = morder[mq * 4 + sub]
                        out_ap = ps_t[:, sub, :]
                        for k in range(KCH):
                            nc.tensor.matmul(
                                out_ap,
                                wsb[:, k, m * 128:(m + 1) * 128],
                                hT[h][:, k, :],
                                start=(k == 0),
                                stop=(k == KCH - 1 and not phase1),
                            )
                        if phase1:
                            nc.tensor.matmul(
                                out_ap,
                                wih[0:1, m * 128:(m + 1) * 128],
                                xst[0:1, :],
                                start=False, stop=True,
                            )
                    for sub in range(4):
                        m = morder[mq * 4 + sub]
                        func = AF.Tanh if m in (4, 5) else AF.Sigmoid
                        ga = gact_pool.tile([128, BH], F32, tag="ga")
                        nc.scalar.activation(ga[:], ps_t[:, sub, :], func,
                                             bias=bias[:, m:m + 1])
                        gacts[m] = ga
                    # hidden chunk p == mq gates complete: c/h update
                    p = mq
                    ig, fg, gg, og = (gacts[0 + p], gacts[2 + p],
                                      gacts[4 + p], gacts[6 + p])
                    nc.vector.tensor_mul(ig[:], ig[:], gg[:])
                    nc.vector.tensor_mul(fg[:], fg[:], c_st[h][:, p, :])
                    nc.vector.tensor_add(c_st[h][:, p, :], ig[:], fg[:])
                    nc.scalar.activation(gg[:], c_st[h][:, p, :], AF.Tanh)
                    nc.vector.tensor_mul(h16[:, p, :], og[:], gg[:])
                    if t < S - 1:
                        nc.sync.dma_start(cc_dst[:, p, :], h16[:, p, :])

                # partial prediction row (W_lin slice . h slice)
                for p in range(2):
                    nc.tensor.matmul(aux[0:1, 1, :], wlin[:, p:p + 1],
                                     h16[:, p, :], start=(p == 0), stop=(p == 1))
                prow = pred_pool.tile([1, BH], F32, tag="pr")
                nc.scalar.copy(prow[:], aux[0:1, 1, :])
                nc.sync.dma_start(preds_d[t:t + 1, boff:boff + BH], prow[:])

                if t == S - 1:
                    return

                nc.gpsimd.collective_compute(
                    "AllGather", mybir.AluOpType.bypass, replica_groups=rg,
                    ins=[cc_in[h][buf]], outs=[cc_out[h][buf]],
                )
                pending_in[h] = cc_out[h][buf].rearrange("(k p) b -> p k b", p=128)

            for rep in range(repeat):
                pending_in[0] = pending_in[1] = None
                for hh in range(NB):
                    nc.vector.memset(hT[hh][:], 0.0)
                    nc.vector.memset(c_st[hh][:], 0.0)
                for t in range(S):
                    for hh in range(NB):
                        emit_half_step(hh, t)

    nc.compile()
    nc.m = get_hw_module(nc.m)
    return nc


def _host_prep(x, W_ih, W_hh, b_ih, b_hh, W_lin, b_lin, T):
    x = np.asarray(x, np.float32)
    W_ih = np.asarray(W_ih, np.float32)
    W_hh = np.asarray(W_hh, np.float32)
    W_lin = np.asarray(W_lin, np.float32)
    b = np.asarray(b_ih, np.float32) + np.asarray(b_hh, np.float32)
    b_lin = float(np.asarray(b_lin).reshape(-1)[0])

    W_eff = (W_hh.astype(np.float64)
             + W_ih.astype(np.float64) @ W_lin.astype(np.float64)).astype(np.float32)
    b_eff = b + b_lin * W_ih[:, 0]

    in_maps = []
    for c in range(NCORES):
        rows = np.concatenate([np.arange(g * H + c * HS, g * H + (c + 1) * HS)
                               for g in range(4)])
        w1 = W_hh[rows, :].T
        w2 = W_eff[rows, :].T
        in_maps.append(dict(
            w1sb=np.ascontiguousarray(
                w1.reshape(KCH, 128, GS).transpose(1, 0, 2)).astype(np.float16),
            w2sb=np.ascontiguousarray(
                w2.reshape(KCH, 128, GS).transpose(1, 0, 2)).astype(np.float16),
            wih=W_ih[rows, 0].reshape(1, GS).astype(np.float16),
            wlin=np.ascontiguousarray(
                W_lin[0, c * HS:(c + 1) * HS].reshape(2, 128).T).astype(np.float16),
            b1=np.ascontiguousarray(b[rows].reshape(MCH, 128).T).astype(np.float32),
            b2=np.ascontiguousarray(b_eff[rows].reshape(MCH, 128).T).astype(np.float32),
            xt16=np.ascontiguousarray(x[:, :T].T).astype(np.float16),
        ))
    return in_maps, b_lin


class _Runner:
    """Reusable jitted PJRT executor for a compiled SPMD module."""

    def __init__(self, nc, n_cores):
        import jax
        from jax.sharding import Mesh, PartitionSpec, NamedSharding
        from jax.experimental.shard_map import shard_map

        install_neuronx_cc_hook()
        self.jax = jax
        partition_name = (nc.partition_id_tensor.name
                          if nc.partition_id_tensor else None)
        self.n_cores = n_cores
        in_names, out_names, out_avals, zero_outs = [], [], [], []
        for alloc in nc.m.functions[0].allocations:
            if not isinstance(alloc, mybir.MemoryLocationSet):
                continue
            name = alloc.memorylocations[0].name
            if alloc.kind == "ExternalInput":
                if name != partition_name:
                    in_names.append(name)
            elif alloc.kind == "ExternalOutput":
                shape = tuple(alloc.tensor_shape)
                dtype = mybir.dt.np(alloc.dtype)
                out_names.append(name)
                out_avals.append(jax.core.ShapedArray(shape, dtype))
                zero_outs.append(np.zeros(shape, dtype))
        self.in_names, self.out_names = in_names, out_names
        self.out_avals, self.zero_outs = out_avals, zero_outs
        n_params, n_outs = len(in_names), len(out_avals)
        all_in_names = in_names + out_names
        if partition_name is not None:
            all_in_names = all_in_names + [partition_name]

        def _body(*args):
            operands = list(args)
            if partition_name is not None:
                operands.append(partition_id_tensor())
            outs = _bass_exec_p.bind(
                *operands,
                out_avals=tuple(out_avals),
                in_names=tuple(all_in_names),
                out_names=tuple(out_names),
                lowering_input_output_aliases=(),
                sim_require_finite=True,
                sim_require_nnan=True,
                nc=nc,
            )
            return tuple(outs)

        devices = jax.devices()[:n_cores]
        assert len(devices) >= 1, "no neuron devices visible"
        self.mesh = Mesh(np.asarray(devices), ("core",))
        self.sh = NamedSharding(self.mesh, PartitionSpec("core"))
        in_specs = (PartitionSpec("core"),) * (n_params + n_outs)
        out_specs = (PartitionSpec("core"),) * n_outs
        donate = tuple(range(n_params, n_params + n_outs))
        self.fn = jax.jit(
            shard_map(_body, mesh=self.mesh, in_specs=in_specs,
                      out_specs=out_specs, check_rep=False),
            donate_argnums=donate, keep_unused=True,
        )
        self._dev_in = None

    def stage_inputs(self, in_maps):
        concat = [np.concatenate([np.asarray(in_maps[c][n])
                                  for c in range(self.n_cores)], axis=0)
                  for n in self.in_names]
        self._dev_in = [self.jax.device_put(a, self.sh) for a in concat]
        self.jax.block_until_ready(self._dev_in)

    def run_results(self):
        zs = [self.jax.device_put(
                  np.zeros((self.n_cores * z.shape[0], *z.shape[1:]), z.dtype),
                  self.sh)
              for z in self.zero_outs]
        out_arrs = self.fn(*self._dev_in, *zs)
        self.jax.block_until_ready(out_arrs)
        res = []
        for c in range(self.n_cores):
            d = {}
            for i, name in enumerate(self.out_names):
                a = np.asarray(out_arrs[i]).reshape(
                    self.n_cores, *self.out_avals[i].shape)
                d[name] = a[c]
            res.append(d)
        return res


_CACHE = {}


def _get_runner(T, FUT, repeat=1):
    key = (T, FUT, repeat)
    if key not in _CACHE:
        nc = _build(T, FUT, repeat=repeat)
        _CACHE[key] = _Runner(nc, NCORES)
    return _CACHE[key]


def kernel(x, W_ih, W_hh, b_ih, b_hh, W_lin, b_lin, future):
    """Full inputs in, full output [256, 160] fp32 out."""
    T = int(np.asarray(x).shape[1])
    FUT = int(future)
    r = _get_runner(T, FUT)
    in_maps, blv = _host_prep(x, W_ih, W_hh, b_ih, b_hh, W_lin, b_lin, T)
    r.stage_inputs(in_maps)
    results = r.run_results()
    acc = np.zeros_like(results[0]["preds"], dtype=np.float64)
    for res in results:
        acc += res["preds"]
    return (acc + blv).T.astype(np.float32)
